# revision 4
# baseline (speedup 1.0000x reference)
"""Trainium2 Bass kernel for nn_MultiHeadSelfAttention_29403346108551.

Reference semantics (faithful to the original nn.Module):
  q/k/v = (x @ W.T + b) .reshape(b, 16, 2048, 64)   # reshape, NOT transpose
  RoPE with a *scalar* position t=seq_len (same angle for every token),
  scores = q k^T / 8, softmax, o = p v, merge heads (real transpose), o @ wo.T + bo.

Key structural facts used for sharding:
  - The head split is a row-major reshape, so head h only touches rows
    [128h, 128h+128) of x, and its 2048 "time" steps are (r, c) -> t = r*16 + c
    with r = row-in-block, c = column-chunk (j // 64).
  - The RoPE rotation is a fixed per-column-pair linear map -> folded into
    wq / wk (and bq / bk) on the host.
  - Core cid handles batch cid//4 and head group cid%4 (4 heads = a contiguous
    512-row slice of x). The row-parallel output projection partials are summed
    on the host during the gather (4 cores per batch), along with bo.

On-device compute (per core), all matmuls in float32r (TF32-like, full PE rate):
  xT [1024, 512] (host-transposed) ->
  Q/K projections into per-head-pair transposed layout qT2/kT2 [128, 2048]
  (partitions = (head parity, d), columns = permuted time t'' = c*128 + r),
  V projection into v_aug [128, 16, 65] (65th column = ones -> softmax
  denominator accumulates for free in the PV matmul),
  S^T = k q^T per (head, chunk, window) -> exp on ACT (scale=1/8 folded in),
  o^T_aug = v_aug^T @ expS  (rows 0-63 = unnormalized o^T, row 64 = denom),
  normalize with DVE reciprocal + broadcast multiply, final projection with
  o2T as the stationary operand, output written in t'' order (host un-permutes).
"""

import numpy as np

import concourse.bass as bass
import concourse.mybir as mybir
import concourse.tile as tile
from concourse import bacc
from concourse.bass_utils import run_bass_kernel_spmd

F32 = mybir.dt.float32
F32R = mybir.dt.float32r

MODEL_DIM = 1024
NUM_HEADS = 16
D_K = 64            # head dim
B = 2
T = 2048
N_CORES = 8
HPC = 4             # heads per core
RPC = 512           # x rows per core
NK = 8              # contraction chunks of 128 over MODEL_DIM
SEQ_POS = 2048      # scalar rope position used by the reference (== seq len)


def _round_fp32r(a: np.ndarray) -> np.ndarray:
    """Round fp32 to the fp32r grid (round-to-nearest-even, 12 low mantissa
    bits dropped) so the PE's fp32r read is exact."""
    b = a.astype(np.float32).view(np.uint32)
    r = (b + 0x7FF + ((b >> 12) & 1)) & np.uint32(0xFFFFF000)
    return r.view(np.float32)


def _build_program() -> bass.Bass:
    nc = bacc.Bacc(None, target_bir_lowering=False, debug=False)

    xT = nc.dram_tensor("xT", [MODEL_DIM, RPC], F32, kind="ExternalInput")
    wqT = nc.dram_tensor("wqT", [MODEL_DIM, MODEL_DIM], F32, kind="ExternalInput")
    wkT = nc.dram_tensor("wkT", [MODEL_DIM, MODEL_DIM], F32, kind="ExternalInput")
    wvT = nc.dram_tensor("wvT", [MODEL_DIM, MODEL_DIM], F32, kind="ExternalInput")
    woT = nc.dram_tensor("woT", [2, 128, MODEL_DIM], F32, kind="ExternalInput")
    bq = nc.dram_tensor("bq", [128, 8], F32, kind="ExternalInput")
    bk = nc.dram_tensor("bk", [128, 8], F32, kind="ExternalInput")
    bv = nc.dram_tensor("bv", [MODEL_DIM], F32, kind="ExternalInput")
    ones16 = nc.dram_tensor("ones16", [16], F32, kind="ExternalInput")
    outp = nc.dram_tensor("outp", [T, MODEL_DIM], F32, kind="ExternalOutput")

    with tile.TileContext(nc) as tc:
        with (
            tc.tile_pool(name="wpool", bufs=16) as wpool,
            tc.tile_pool(name="wopool", bufs=2) as wopool,
            tc.tile_pool(name="xpool", bufs=8) as xpool,
            tc.tile_pool(name="qkpool", bufs=4) as qkpool,
            tc.tile_pool(name="vpool", bufs=4) as vpool,
            tc.tile_pool(name="espool", bufs=4) as espool,
            tc.tile_pool(name="o2pool", bufs=2) as o2pool,
            tc.tile_pool(name="outpool", bufs=2) as outpool,
            tc.tile_pool(name="cpool", bufs=1) as cpool,
            tc.tile_pool(name="rcpool", bufs=2) as rcpool,
            tc.tile_pool(name="psproj", bufs=2, space="PSUM") as psproj,
            tc.tile_pool(name="psS", bufs=2, space="PSUM") as psS_pool,
            tc.tile_pool(name="psO", bufs=1, space="PSUM") as psO_pool,
        ):
            # ---- constant / input loads ----
            xt = []
            for k in range(NK):
                t_ = xpool.tile([128, RPC], F32R, tag="xt", name=f"xt_{k}")
                nc.sync.dma_start(out=t_, in_=xT[k * 128:(k + 1) * 128, :].bitcast(F32R))
                xt.append(t_)

            wq_sb, wk_sb = [], []
            for k in range(NK):
                t_ = wpool.tile([128, MODEL_DIM], F32R, tag="w", name=f"wq_{k}")
                nc.sync.dma_start(out=t_, in_=wqT[k * 128:(k + 1) * 128, :].bitcast(F32R))
                wq_sb.append(t_)
            for k in range(NK):
                t_ = wpool.tile([128, MODEL_DIM], F32R, tag="w", name=f"wk_{k}")
                nc.sync.dma_start(out=t_, in_=wkT[k * 128:(k + 1) * 128, :].bitcast(F32R))
                wk_sb.append(t_)

            bq_sb = cpool.tile([128, 8], F32)
            nc.sync.dma_start(out=bq_sb, in_=bq[:, :])
            bk_sb = cpool.tile([128, 8], F32)
            nc.sync.dma_start(out=bk_sb, in_=bk[:, :])
            # bv broadcast to all 128 partitions via partition-step-0 DMA
            bv_bc = cpool.tile([128, MODEL_DIM], F32)
            bv_ap = bv[:]
            bv_bcast_src = bass.AP(
                tensor=bv_ap.tensor, offset=bv_ap.offset,
                ap=[[0, 128]] + [list(p) for p in bv_ap.ap],
            )
            nc.sync.dma_start(out=bv_bc, in_=bv_bcast_src)

            # ---- Q / K projections ----
            # psQ[64*half + d, rr] = q_pre[j = 128p + 64*half + d, row rr]
            qT2 = [qkpool.tile([128, T], F32R, tag="qk", name=f"qT2_{i}") for i in range(2)]
            kT2 = [qkpool.tile([128, T], F32R, tag="qk", name=f"kT2_{i}") for i in range(2)]

            for w_sb, bias_sb, dst in ((wq_sb, bq_sb, qT2), (wk_sb, bk_sb, kT2)):
                for p in range(8):
                    ps = psproj.tile([128, RPC], F32, tag="proj")
                    for k in range(NK):
                        nc.tensor.matmul(
                            ps, w_sb[k][:, p * 128:(p + 1) * 128], xt[k],
                            start=(k == 0), stop=(k == NK - 1),
                        )
                    for bl in range(HPC):
                        for half in range(2):
                            c = 2 * p + half
                            m, ph = bl // 2, bl % 2
                            nc.vector.tensor_scalar_add(
                                dst[m][64 * ph:64 * ph + 64, c * 128:(c + 1) * 128],
                                ps[64 * half:64 * half + 64, bl * 128:(bl + 1) * 128],
                                bias_sb[64 * half:64 * half + 64, p:p + 1],
                            )

            # ---- V projection (natural layout + ones column) ----
            wv_sb = []
            for k in range(NK):
                t_ = wpool.tile([128, MODEL_DIM], F32R, tag="w", name=f"wv_{k}")
                nc.sync.dma_start(out=t_, in_=wvT[k * 128:(k + 1) * 128, :].bitcast(F32R))
                wv_sb.append(t_)

            v_aug = []
            for bl in range(HPC):
                va = vpool.tile([128, 16, 65], F32R, tag="va", name=f"v_aug_{bl}")
                ones_ap = ones16[:]
                nc.sync.dma_start(
                    out=va[:, :, 64:65],
                    in_=bass.AP(
                        tensor=ones_ap.tensor, offset=ones_ap.offset,
                        ap=[[0, 128]] + [list(p) for p in ones_ap.ap],
                    ).bitcast(F32R),
                )
                v_aug.append(va)
            for bl in range(HPC):
                for jw in range(2):
                    ps = psproj.tile([128, RPC], F32, tag="proj")
                    for k in range(NK):
                        nc.tensor.matmul(
                            ps, xt[k][:, bl * 128:(bl + 1) * 128],
                            wv_sb[k][:, jw * 512:(jw + 1) * 512],
                            start=(k == 0), stop=(k == NK - 1),
                        )
                    for cc in range(8):
                        c = 8 * jw + cc
                        nc.vector.tensor_tensor(
                            v_aug[bl][:, c, 0:64],
                            ps[:, cc * 64:(cc + 1) * 64],
                            bv_bc[:, c * 64:c * 64 + 64],
                            mybir.AluOpType.add,
                        )

            # ---- attention per head ----
            wo_sb = []
            for m_ in range(2):
                t_ = wopool.tile([128, MODEL_DIM], F32R, tag="wo", name=f"wo_{m_}")
                nc.sync.dma_start(out=t_, in_=woT[m_, :, :].bitcast(F32R))
                wo_sb.append(t_)

            o2T = [o2pool.tile([128, T], F32R, tag="o2", name=f"o2T_{i}") for i in range(2)]

            for h in range(HPC):
                m, ph = h // 2, h % 2
                base = 64 * ph
                psO = psO_pool.tile([65, T], F32, tag="o")
                for c in range(16):
                    for w in range(4):
                        psS = psS_pool.tile([128, 512], F32, tag="s")
                        nc.tensor.matmul(
                            psS,
                            kT2[m][base:base + 64, c * 128:(c + 1) * 128],
                            qT2[m][base:base + 64, w * 512:(w + 1) * 512],
                            start=True, stop=True,
                        )
                        eS = espool.tile([128, 512], F32R, tag="es")
                        nc.scalar.activation(
                            eS, psS, mybir.ActivationFunctionType.Exp, scale=0.125,
                        )
                        nc.tensor.matmul(
                            psO[:, w * 512:(w + 1) * 512],
                            v_aug[h][:, c, :], eS,
                            start=(c == 0), stop=(c == 15),
                        )
                rcp = rcpool.tile([1, T], F32, tag="rcp")
                nc.vector.reciprocal(rcp, psO[64:65, :])
                rcp_bc = rcpool.tile([64, T], F32, tag="rcpb")
                nc.gpsimd.partition_broadcast(rcp_bc, rcp)
                nc.vector.tensor_tensor(
                    o2T[m][base:base + 64, :],
                    psO[0:64, :],
                    rcp_bc,
                    mybir.AluOpType.mult,
                )

            # ---- final projection (t'' order; host un-permutes rows) ----
            for tt in range(16):
                out_sb = outpool.tile([128, MODEL_DIM], F32, tag="out")
                for jw in range(2):
                    ps = psproj.tile([128, 512], F32, tag="proj")
                    for m_ in range(2):
                        nc.tensor.matmul(
                            ps,
                            o2T[m_][:, tt * 128:(tt + 1) * 128],
                            wo_sb[m_][:, jw * 512:(jw + 1) * 512],
                            start=(m_ == 0), stop=(m_ == 1),
                        )
                    nc.scalar.copy(out_sb[:, jw * 512:(jw + 1) * 512], ps)
                nc.sync.dma_start(out=outp[tt * 128:(tt + 1) * 128, :], in_=out_sb)

    nc.compile()
    return nc


_NC_CACHE = None


def _get_program():
    global _NC_CACHE
    if _NC_CACHE is None:
        _NC_CACHE = _build_program()
    return _NC_CACHE


def _host_prep(inputs):
    x = np.asarray(inputs["x"], np.float32)
    wq = np.asarray(inputs["wq"], np.float32)
    wk = np.asarray(inputs["wk"], np.float32)
    wv = np.asarray(inputs["wv"], np.float32)
    wo = np.asarray(inputs["wo"], np.float32)
    bq = np.asarray(inputs["bq"], np.float32)
    bk = np.asarray(inputs["bk"], np.float32)
    bv = np.asarray(inputs["bv"], np.float32)
    rot_cos = np.asarray(inputs["rot_cos"], np.float32)
    rot_sin = np.asarray(inputs["rot_sin"], np.float32)

    cos = rot_cos[SEQ_POS]  # [32]
    sin = rot_sin[SEQ_POS]

    def rope_fold_w(w):
        # q_rot = R @ q with 2x2 blocks per (group, k) pair of output rows
        wv_ = w.reshape(16, 32, 2, MODEL_DIM)
        ev = wv_[:, :, 0] * cos[None, :, None] - wv_[:, :, 1] * sin[None, :, None]
        od = wv_[:, :, 0] * sin[None, :, None] + wv_[:, :, 1] * cos[None, :, None]
        return np.stack([ev, od], axis=2).reshape(MODEL_DIM, MODEL_DIM)

    def rope_fold_b(b_):
        bv_ = b_.reshape(16, 32, 2)
        ev = bv_[:, :, 0] * cos[None, :] - bv_[:, :, 1] * sin[None, :]
        od = bv_[:, :, 0] * sin[None, :] + bv_[:, :, 1] * cos[None, :]
        return np.stack([ev, od], axis=2).reshape(MODEL_DIM)

    wq_r = rope_fold_w(wq)
    wk_r = rope_fold_w(wk)
    bq_r = rope_fold_b(bq)
    bk_r = rope_fold_b(bk)

    wqT = _round_fp32r(np.ascontiguousarray(wq_r.T))
    wkT = _round_fp32r(np.ascontiguousarray(wk_r.T))
    wvT = _round_fp32r(np.ascontiguousarray(wv.T))
    bq_sb = np.ascontiguousarray(bq_r.reshape(8, 128).T)
    bk_sb = np.ascontiguousarray(bk_r.reshape(8, 128).T)

    in_maps = []
    for cid in range(N_CORES):
        bi, g = cid // 4, cid % 4
        xTc = _round_fp32r(np.ascontiguousarray(x[bi, 512 * g:512 * (g + 1), :].T))
        woTc = np.stack(
            [
                np.ascontiguousarray(
                    wo[:, (4 * g + 2 * m) * 64:(4 * g + 2 * m + 2) * 64].T
                )
                for m in range(2)
            ]
        )
        in_maps.append({
            "xT": xTc,
            "wqT": wqT, "wkT": wkT, "wvT": wvT,
            "woT": _round_fp32r(woTc),
            "bq": bq_sb, "bk": bk_sb, "bv": bv,
            "ones16": np.ones(16, np.float32),
        })
    return in_maps, np.asarray(inputs["bo"], np.float32)


def _gather(results, bo):
    out = np.empty((B, T, MODEL_DIM), np.float32)
    for bi in range(B):
        acc = results[4 * bi]["outp"].astype(np.float32).copy()
        for g in range(1, 4):
            acc += results[4 * bi + g]["outp"]
        # t'' = c*128 + r  ->  t = r*16 + c
        acc = acc.reshape(16, 128, MODEL_DIM).transpose(1, 0, 2).reshape(T, MODEL_DIM)
        out[bi] = acc + bo[None, :]
    return out


def _run(inputs, trace=False, **kw):
    nc = _get_program()
    in_maps, bo = _host_prep(inputs)
    res = run_bass_kernel_spmd(nc, in_maps, list(range(N_CORES)), trace=trace, **kw)
    return _gather(res.results, bo), res


def kernel(**inputs) -> np.ndarray:
    out, _ = _run(inputs)
    return out
